# revision 44
# baseline (speedup 1.0000x reference)
"""Multi-head attention with 2D RoPE on 8 Trainium2 NeuronCores.

Problem (hardcoded): B=8, L=1024, EMB=768, 12 heads x 64 dim, 2D RoPE
(x/y tables of length 32, base 100), softmax attention, output projection.

Sharding: data-parallel over batch — one batch element per core, no
collectives.

Per-core kernel structure (v2 — PE-roofline oriented):

    qT/kT = (W/pair)^T @ embT in fp32r, rope via elementwise cos/ssh plus a
        16-lane swap as a PE matmul (128x128 permutation, in-place PSUM)
    per (pair, q-half 512, key-tile j):
        sT = kT^T @ qT        [128 keys, 2 heads x 512 q], tile_position row
                              packing, fp32r, ap 512 (output-bandwidth floor)
        expT = exp(sT) bf16   (ACT, no max-subtraction: |scores| <~ 6)
        AV FLIPPED: stationary = expT [128k x 128q] slices (ldweights is
            free), moving = v tile bf16 [128 x 64] -> av2[q, d] in PSUM.
            49.9k PE cycles vs 98.3k for the moving-expT orientation.
        sums = expT^T @ ones  (ap-1 matmuls into a shared [128, 96] bank)
        All sub-bank accumulation regions share one PSUM zero-region, so
        banks are Pool-memset to 0 and every matmul uses start=False with
        skip_group_check (HW accumulates onto existing content).
    normalize: r = 1/sums per (q, head) on DVE; Pool scale-copies
        av2 -> avsb bf16 (per-partition scalar = per-query, native)
    transpose avsb [q, chan] -> avT [chan, q] via XBAR DMA transpose
        (14ns per 16x128 tile; no PE cost)
    out = attnout @ Wp(bf16) + bp, K=128 accumulation over pairs

Engine budget (TimelineSim model): PE ~309k cycles @2.4GHz ~= 129us is the
bottleneck; ACT exp ~100us; DVE (rope) ~57us; Pool (copies/normalize)
~33us; DMA ~49us. Next-pair projections interleave into the j-loops so PE
never starves behind the ACT exp stream; AV emission lags scores by 2
j-tiles so PE never head-blocks on an exp dependency (engine queues are
strictly in-order).
"""

import numpy as np

import concourse.bass as bass
import concourse.mybir as mybir
import concourse.tile as tile
from concourse import bacc
from concourse.bass import ts
from concourse.bass_utils import run_bass_kernel_spmd

F32 = mybir.dt.float32
F32R = mybir.dt.float32r
BF16 = mybir.dt.bfloat16
AF = mybir.ActivationFunctionType

HEAD_NUM = 12
EMB = 768
HEAD = 64
L = 1024
B = 8
X_SIZE = 32
Y_SIZE = 32
BASE = 100.0
N_CORES = 8

KT = EMB // 128   # 6 contraction tiles over channels
NJ = L // 128     # 8 key tiles
NPAIR = HEAD_NUM // 2  # 6 head pairs


def _round_f32r(x):
    """Round fp32 to FP22 (e8m13, drop 10 mantissa LSBs, RNE)."""
    v = np.ascontiguousarray(x, dtype=np.float32).view(np.uint32).copy()
    v = v + (np.uint32(0x1FF) + ((v >> np.uint32(10)) & np.uint32(1)))
    v &= np.uint32(0xFFFFFC00)
    return v.view(np.float32)


def _tables_np(pos_len, d, base=BASE):
    inv_freq = 1.0 / base ** (np.arange(0, d, 2, dtype=np.float32) / d)
    freqs = np.outer(np.arange(pos_len, dtype=np.float32), inv_freq)
    freqs = np.concatenate([freqs, freqs], axis=-1)
    return np.sin(freqs).astype(np.float32), np.cos(freqs).astype(np.float32)


def _rope_coeffs(pos):
    """cos128/ssh128: [128, L] elementwise RoPE coefficients, 2 heads deep.

    Row layout per 64-row head block: rows 0:32 x-part, rows 32:64 y-part.
    ssh is the sin table pre-shifted/negated so that
        rope(q) = q * cos128 + R128 @ (q * ssh128)
    where R128 swaps 16-row halves within each 32-row block.
    """
    sx, cx = _tables_np(X_SIZE, HEAD // 2)
    sy, cy = _tables_np(Y_SIZE, HEAD // 2)
    px, py = pos[:, 0], pos[:, 1]
    cosxT = cx[px].T  # [32, L]
    cosyT = cy[py].T
    sinxT = sx[px].T
    sinyT = sy[py].T

    def shift(s):
        out = np.empty_like(s)
        out[0:16] = s[16:32]
        out[16:32] = -s[0:16]
        return out

    cos64 = np.concatenate([cosxT, cosyT], axis=0)          # [64, L]
    ssh64 = np.concatenate([shift(sinxT), shift(sinyT)], axis=0)
    cos128 = np.concatenate([cos64, cos64], axis=0).astype(np.float32)
    ssh128 = np.concatenate([ssh64, ssh64], axis=0).astype(np.float32)
    return np.ascontiguousarray(cos128), np.ascontiguousarray(ssh128)


def _r128():
    r32 = np.zeros((32, 32), dtype=np.float32)
    for d in range(16):
        r32[d, d + 16] = 1.0
        r32[d + 16, d] = 1.0
    return np.kron(np.eye(4, dtype=np.float32), r32)


def build_nc(debug=False):
    nc = bacc.Bacc()
    embT = nc.declare_dram_parameter("embT", [EMB, L], F32R, isOutput=False)
    wqs = nc.declare_dram_parameter("wqs", [NPAIR, 128, EMB], F32R,
                                    isOutput=False)
    wks = nc.declare_dram_parameter("wks", [NPAIR, 128, EMB], F32R,
                                    isOutput=False)
    wv = nc.declare_dram_parameter("wv", [EMB, EMB], F32R, isOutput=False)
    wp = nc.declare_dram_parameter("wp", [EMB, EMB], BF16, isOutput=False)
    bp = nc.declare_dram_parameter("bp", [1, EMB], F32, isOutput=False)
    cos = nc.declare_dram_parameter("cos", [128, L], F32, isOutput=False)
    ssh = nc.declare_dram_parameter("ssh", [128, L], F32, isOutput=False)
    r128 = nc.declare_dram_parameter("r128", [128, 128], F32R, isOutput=False)
    ident = nc.declare_dram_parameter("ident", [128, 128], BF16,
                                      isOutput=False)
    out = nc.declare_dram_parameter("out", [L, EMB], F32, isOutput=True)
    if debug:
        d_qT = nc.declare_dram_parameter("d_qT", [128, L], F32, isOutput=True)
        d_kT = nc.declare_dram_parameter("d_kT", [128, L], F32, isOutput=True)
        d_exp = nc.declare_dram_parameter("d_exp", [128, L], BF16,
                                          isOutput=True)
        d_vaug = nc.declare_dram_parameter("d_vaug", [128, EMB], BF16,
                                           isOutput=True)
        d_avsb = nc.declare_dram_parameter("d_avsb", [128, 4, 128], BF16,
                                           isOutput=True)
        d_avsb2 = nc.declare_dram_parameter("d_avsb2", [128, 4, 128], BF16,
                                            isOutput=True)
        d_avT0h1 = nc.declare_dram_parameter("d_avT0h1", [128, 4, 128], BF16,
                                             isOutput=True)
        d_rsb = nc.declare_dram_parameter("d_rsb", [128, 8], F32,
                                          isOutput=True)
        d_avT = nc.declare_dram_parameter("d_avT", [NPAIR, 128, NJ, 128],
                                          BF16, isOutput=True)
        d_sums = nc.declare_dram_parameter("d_sums", [128, 96], F32,
                                           isOutput=True)
        d_rsb2 = nc.declare_dram_parameter("d_rsb2", [128, 8], F32,
                                           isOutput=True)
        d_av2 = nc.declare_dram_parameter("d_av2", [128, 512], F32,
                                          isOutput=True)

    with tile.TileContext(nc) as tc:
        with (
            tc.tile_pool(name="const", bufs=1) as p_const,
            tc.tile_pool(name="vaug", bufs=1) as p_vaug,
            tc.tile_pool(name="persist", bufs=1) as p_per,
            tc.tile_pool(name="wsl", bufs=2) as p_wsl,
            tc.tile_pool(name="qk", bufs=2) as p_qk,
            tc.tile_pool(name="tmp", bufs=2) as p_tmp,
            tc.tile_pool(name="exp", bufs=4) as p_exp,
            tc.tile_pool(name="avsb", bufs=2) as p_avsb,
            tc.tile_pool(name="rsb", bufs=2) as p_rsb,
            tc.tile_pool(name="outp", bufs=2) as p_out,
            tc.tile_pool(name="big", bufs=2, space="PSUM") as ps_big,
            tc.tile_pool(name="qp", bufs=1, space="PSUM") as ps_qp,
            tc.tile_pool(name="av", bufs=2, space="PSUM") as ps_av,
            tc.tile_pool(name="sum", bufs=1, space="PSUM") as ps_sum,
        ):
            # attnout^T per pair: [chan 128, qtile 8, q 128] bf16.
            # Allocated FIRST so the XBAR-transpose destinations sit at the
            # lowest SBUF addresses.
            avT_t = [p_per.tile([128, NJ, 128], BF16, tag=f"avT{p}",
                                name=f"avT{p}") for p in range(NPAIR)]

            # weight slices are pre-swizzled on the host to the exact
            # [128, KT*128] SBUF layout, so each load is one contiguous DMA
            def load_wslice(w_dram, pair, wtag):
                wsl = p_wsl.tile([128, EMB], F32R, tag=wtag,
                                 name=f"wsl{wtag}{pair}")
                nc.sync.dma_start(wsl[:], w_dram[pair])
                return wsl

            # ---- loads: pair-0 q slice + embT first, then wv (v-proj is
            # interleaved into pair 0's first j-loop), rope coeffs last of
            # the startup-critical set ----
            PRE_Q0 = load_wslice(wqs, 0, "q")

            # PE warmup: throwaway matmuls during the DMA head keep the HAM
            # clock-gate warm; lands in an av-pool slot (group closes, the
            # slot is memset before real use)
            wu = p_const.tile([128, 512], F32, tag="warm")
            nc.gpsimd.memset(wu[:], 0.0)
            wup = ps_av.tile([128, 512], F32, tag="av2", name="warmps")
            for _ in range(2):
                nc.tensor.matmul(wup[0:64, :], wu[:, 0:64], wu[:],
                                 start=True, stop=True)

            embT_t = [p_per.tile([128, L], F32R, tag=f"embT{k}",
                                 name=f"embTt{k}") for k in range(KT)]
            for k in range(KT):
                eng = nc.sync if k % 2 == 0 else nc.scalar
                eng.dma_start(embT_t[k][:], embT[ts(k, 128), :])
            PRE_K0 = load_wslice(wks, 0, "k")
            wv_t = [p_per.tile([128, EMB], F32R, tag=f"wvp{k}",
                               name=f"wvt{k}") for k in range(KT)]
            for k in range(KT):
                nc.sync.dma_start(wv_t[k][:], wv[ts(k, 128), :])
            cos_t = p_const.tile([128, L], F32, tag="cos")
            ssh_t = p_const.tile([128, L], F32, tag="ssh")
            r_t = p_const.tile([128, 128], F32R, tag="r128")
            nc.sync.dma_start(cos_t[:], cos[:])
            nc.sync.dma_start(ssh_t[:], ssh[:])
            nc.sync.dma_start(r_t[:], r128[:])

            # ones moving-vector for the ap-1 softmax-denominator matmuls
            ones_mv = p_const.tile([128, 1], BF16, tag="ones")
            nc.gpsimd.memset(ones_mv[:], 1.0)
            # bf16 identity for PE transposes
            id_t = p_const.tile([128, 128], BF16, tag="ident")
            nc.sync.dma_start(id_t[:], ident[:])
            # zero moving tensor: accumulator banks that hold many sub-bank
            # regions are zeroed by one full-bank start=True matmul (a
            # regular matmul's start zeroes its whole 2KB zero-region), then
            # every accumulate uses start=False + skip_group_check
            zero_f = p_const.tile([128, 512], F32, tag="zmv")
            nc.gpsimd.memset(zero_f[:], 0.0)

            # one shared PSUM bank of [128, 1] denominator accumulators,
            # region col = pair*16 + half*8 + qs*2 + head
            sums_ps = ps_sum.tile([128, 96], F32, tag="sums")
            nc.tensor.matmul(sums_ps[:], zero_f[:, 0:128].bitcast(F32R),
                             zero_f[:, 0:96].bitcast(F32R),
                             start=True, stop=True)

            vaug_t = [p_vaug.tile([128, EMB], BF16, tag=f"vaug{j}",
                                  name=f"vaug{j}")
                      for j in range(NJ)]

            # rope'd projection: two 512-col chunks through the single
            # [128, 512] qp PSUM bank; chunk emission is interleaved into
            # the attention j-loops via the returned closure
            def make_proj(pair, wtag, wsl):
                dst = p_qk.tile([128, L], F32R, tag=wtag,
                                name=f"{wtag}T{pair}")

                def chunk(ci):
                    c0 = ci * 512
                    qp = ps_qp.tile([128, 512], F32, tag="qp",
                                    name=f"qp{wtag}{pair}{ci}")
                    for k in range(KT):
                        nc.tensor.matmul(
                            qp[:],
                            wsl[:, ts(k, 128)],
                            embT_t[k][:, c0:c0 + 512],
                            start=(k == 0), stop=(k == KT - 1),
                        )
                    t_s = p_tmp.tile([128, 512], F32R, tag="ts")
                    t_c = p_tmp.tile([128, 512], F32, tag="tc")
                    nc.vector.tensor_mul(t_s[:], qp[:], ssh_t[:, c0:c0 + 512])
                    nc.vector.tensor_mul(t_c[:], qp[:], cos_t[:, c0:c0 + 512])
                    nc.tensor.matmul(qp[:], r_t[:], t_s[:],
                                     start=True, stop=True)
                    nc.vector.tensor_add(dst[:, c0:c0 + 512], t_c[:], qp[:])
                return dst, chunk

            # v projection for one key tile -> vaug[j] (bf16); channel
            # layout already matches the flipped-AV moving operand
            def project_v(j):
                vp = ps_big.tile([128, L], F32, tag="big", name=f"vp{j}")
                for c0, c1 in ((0, 512), (512, 768)):
                    for k in range(KT):
                        nc.tensor.matmul(
                            vp[:, c0:c1],
                            embT_t[k][:, ts(j, 128)],
                            wv_t[k][:, c0:c1],
                            start=(k == 0), stop=(k == KT - 1),
                        )
                nc.scalar.copy(vaug_t[j][:], vp[:, 0:EMB])

            # pair-0 projections (DMA-paced startup)
            qT, q_chunk = make_proj(0, "q", PRE_Q0)
            q_chunk(0)
            q_chunk(1)
            kT, k_chunk = make_proj(0, "k", PRE_K0)
            k_chunk(0)
            k_chunk(1)

            # remaining weight loads (after the startup-critical set)
            wp_t = [p_per.tile([128, EMB], BF16, tag=f"wpp{k}",
                               name=f"wpt{k}") for k in range(KT)]
            for k in range(KT):
                nc.sync.dma_start(wp_t[k][:], wp[ts(k, 128), :])
            bpb_t = p_const.tile([128, EMB], F32, tag="bpb")
            nc.sync.dma_start(bpb_t[:], bp[:].to_broadcast((128, EMB)))

            # final projection for one 128-query tile (+bias, store)
            def fp_qtile(qt):
                fp = ps_big.tile([128, L], F32, tag="big", name=f"fp{qt}")
                for c0, c1 in ((0, 512), (512, 768)):
                    for pp in range(NPAIR):
                        nc.tensor.matmul(
                            fp[:, c0:c1],
                            avT_t[pp][:, qt],
                            wp_t[pp][:, c0:c1],
                            start=(pp == 0), stop=(pp == NPAIR - 1),
                        )
                o_sb = p_out.tile([128, EMB], F32, tag="osb",
                                  name=f"osb{qt}")
                nc.vector.tensor_add(o_sb[:], fp[:, 0:EMB], bpb_t[:])
                oeng = nc.sync if qt % 2 == 0 else nc.scalar
                oeng.dma_start(out[ts(qt, 128), :], o_sb[:])

            for pair in range(NPAIR):
                if pair + 1 < NPAIR:
                    wsl_q = load_wslice(wqs, pair + 1, "q")
                    wsl_k = load_wslice(wks, pair + 1, "k")
                    qT_n, qn_chunk = make_proj(pair + 1, "q", wsl_q)
                    kT_n, kn_chunk = make_proj(pair + 1, "k", wsl_k)
                for half in (0, 1):
                    av2 = ps_av.tile([128, 512], F32, tag="av2",
                                     name=f"av2_{pair}_{half}")
                    nc.tensor.matmul(av2[:], zero_f[:, 0:128].bitcast(F32R),
                                     zero_f[:].bitcast(F32R),
                                     start=True, stop=True)

                    # flipped AV + denominator matmuls for key tile j
                    def emit_av(j, expt):
                        for qs in range(4):
                            for hh in range(2):
                                e_sl = expt[:, 512 * hh + 128 * qs:
                                            512 * hh + 128 * qs + 128]
                                nc.tensor.matmul(
                                    av2[:, qs * 128 + hh * 64:
                                        qs * 128 + hh * 64 + 64],
                                    e_sl,
                                    vaug_t[j][:, (2 * pair + hh) * 64:
                                              (2 * pair + hh) * 64 + 64],
                                    start=False, stop=(j == NJ - 1),
                                    skip_group_check=True,
                                )
                                sc = pair * 16 + half * 8 + qs * 2 + hh
                                nc.tensor.matmul(
                                    sums_ps[:, sc:sc + 1],
                                    e_sl,
                                    ones_mv[:],
                                    start=False, stop=(j == NJ - 1),
                                    skip_group_check=True,
                                )

                    pend = []
                    for j in range(NJ):
                        if len(pend) >= 2:
                            emit_av(*pend.pop(0))
                        if pair == 0 and half == 0:
                            project_v(j)
                        sAB = ps_big.tile([128, L], F32, tag="big",
                                          name=f"s{pair}_{half}_{j}")
                        for hh in range(2):
                            p0 = 64 * hh
                            nc.tensor.matmul(
                                sAB[:, 512 * hh:512 * hh + 512],
                                kT[p0:p0 + 64, ts(j, 128)],
                                qT[p0:p0 + 64, 512 * half:512 * half + 512],
                                start=True, stop=True,
                                tile_position=(p0, 0),
                            )
                        expt = p_exp.tile([128, L], BF16, tag="expt",
                                          name=f"e{pair}_{half}_{j}")
                        nc.scalar.activation(expt[:], sAB[:], AF.Exp)
                        if debug and pair == 0 and half == 0 and j == 0:
                            nc.sync.dma_start(d_exp[:], expt[:])
                            nc.sync.dma_start(d_vaug[:], vaug_t[0][:])
                        pend.append((j, expt))
                        # interleave next-pair projection chunks (q during
                        # half 0, k during half 1) so the PE excursion lands
                        # while ACT still has queued exps
                        if pair + 1 < NPAIR:
                            if j == 2:
                                (qn_chunk if half == 0 else kn_chunk)(0)
                            elif j == 5:
                                (qn_chunk if half == 0 else kn_chunk)(1)
                        # last pair, half 1: interleave the final projection
                        # of the ready half-0 query tiles
                        if pair == NPAIR - 1 and half == 1 and j in (1, 3, 5, 7):
                            fp_qtile(j // 2)
                    for item in pend:
                        emit_av(*item)
                    pend.clear()

                    # normalize: per-(query, head) reciprocal, then Pool
                    # scale-copies into the bf16 transpose staging tile
                    soff = pair * 16 + half * 8
                    r_sb = p_rsb.tile([128, 8], F32, tag="rsb",
                                      name=f"r{pair}_{half}")
                    nc.vector.reciprocal_approx_fast(
                        r_sb[:], sums_ps[:, soff:soff + 8])
                    avsb = p_avsb.tile([128, 4, 128], BF16, tag="avsb",
                                       name=f"avsb{pair}_{half}")
                    for qs in range(4):
                        for hh in range(2):
                            nc.vector.tensor_scalar_mul(
                                avsb[:, qs, hh * 64:hh * 64 + 64],
                                av2[:, qs * 128 + hh * 64:
                                    qs * 128 + hh * 64 + 64],
                                r_sb[:, qs * 2 + hh:qs * 2 + hh + 1])
                    if debug and pair == 0 and half == 0:
                        nc.sync.dma_start(d_avsb[:], avsb[:])
                        nc.sync.dma_start(d_rsb[:], r_sb[:])
                        nc.scalar.dma_start(d_qT[:], qT[:].bitcast(F32))
                        nc.scalar.dma_start(d_kT[:], kT[:].bitcast(F32))
                    if debug and pair == 1 and half == 0:
                        nc.sync.dma_start(d_rsb2[:], r_sb[:])
                        av2c = p_const.tile([128, 512], F32, tag="dav2")
                        nc.vector.tensor_copy(av2c[:], av2[:])
                        nc.sync.dma_start(d_av2[:], av2c[:])
                        sumsc = p_const.tile([128, 96], F32, tag="dsums")
                        nc.vector.tensor_copy(sumsc[:], sums_ps[:])
                        nc.sync.dma_start(d_sums[:], sumsc[:])
                    # PE transpose [q, chan] -> [chan, q] per qtile (the
                    # XBAR DMA transpose corrupts under concurrent DMA
                    # traffic), then DVE drains PSUM -> avT in SBUF
                    tp = ps_big.tile([128, L], F32, tag="big",
                                     name=f"tp{pair}_{half}")
                    for qs in range(4):
                        sub = tp[:, 128 * qs:128 * qs + 64].bitcast(BF16)
                        nc.tensor.transpose(sub, avsb[:, qs, :], id_t[:])
                        nc.vector.tensor_copy(
                            avT_t[pair][:, 4 * half + qs, :], sub)
                    if debug and pair == 0 and half == 1:
                        # dumped AFTER the transpose so its emission does not
                        # delay the transpose (avoids masking the race)
                        nc.scalar.dma_start(d_avsb2[:], avsb[:])
                        nc.scalar.dma_start(
                            d_avT0h1[:], avT_t[pair][:, 4:8, :])
                if pair + 1 < NPAIR:
                    qT, kT = qT_n, kT_n

            for qt in range(4, NJ):
                fp_qtile(qt)
            if debug:
                for p in range(NPAIR):
                    nc.sync.dma_start(d_avT[p], avT_t[p][:])

    nc.finalize()
    return nc


_NC_CACHE = {}


def _get_nc(variant=None):
    if variant not in _NC_CACHE:
        _NC_CACHE[variant] = build_nc(debug=(variant == "debug"))
    return _NC_CACHE[variant]


def kernel(emb, pos, Wq, Wk, Wv, Wp, bp, _trace=False, _cores=N_CORES,
           _debug=False):
    import ml_dtypes

    emb = np.asarray(emb, dtype=np.float32)
    pos = np.asarray(pos)
    Wq_s = _round_f32r(np.asarray(Wq, dtype=np.float32) * (HEAD ** -0.5))
    Wk_r = _round_f32r(np.asarray(Wk, dtype=np.float32))
    Wv_r = _round_f32r(np.asarray(Wv, dtype=np.float32))
    Wp_b = np.asarray(Wp, dtype=np.float32).astype(ml_dtypes.bfloat16)
    bp2 = np.asarray(bp, dtype=np.float32).reshape(1, EMB)

    cos128, ssh128 = _rope_coeffs(np.asarray(pos))
    r128 = _r128()
    ident = np.eye(128, dtype=np.float32).astype(ml_dtypes.bfloat16)

    def swizzle(w):
        # [EMB, EMB] -> [NPAIR, 128, KT*128]: slice pair columns, gather
        # row t*128+p to partition p, k-tile-major free layout
        return np.ascontiguousarray(
            w.reshape(KT, 128, NPAIR, 128).transpose(2, 1, 0, 3)
            .reshape(NPAIR, 128, EMB))

    nc = _get_nc("debug" if _debug else None)
    wqs = swizzle(Wq_s)
    wks = swizzle(Wk_r)
    in_maps = []
    for b in range(_cores):
        in_maps.append({
            "embT": _round_f32r(emb[b].T),
            "wqs": wqs, "wks": wks, "wv": Wv_r, "wp": Wp_b, "bp": bp2,
            "cos": cos128, "ssh": ssh128, "r128": r128, "ident": ident,
        })
    res = run_bass_kernel_spmd(nc, in_maps, list(range(_cores)), trace=_trace)
    out = np.stack([res.results[b]["out"] for b in range(_cores)], axis=0)
    if _debug:
        return out, res.results
    if _trace:
        return out, res
    return out


# revision 49
# speedup vs baseline: 1.1219x; 1.1219x over previous
"""Multi-head attention with 2D RoPE on 8 Trainium2 NeuronCores.

Problem (hardcoded): B=8, L=1024, EMB=768, 12 heads x 64 dim, 2D RoPE
(x/y tables of length 32, base 100), softmax attention, output projection.

Sharding: data-parallel over batch — one batch element per core, no
collectives.

Per-core kernel structure (v2 — PE-roofline oriented):

    qT/kT = (W/pair)^T @ embT in fp32r, rope via elementwise cos/ssh plus a
        16-lane swap as a PE matmul (128x128 permutation, in-place PSUM)
    per (pair, q-half 512, key-tile j):
        sT = kT^T @ qT        [128 keys, 2 heads x 512 q], tile_position row
                              packing, fp32r, ap 512 (output-bandwidth floor)
        expT = exp(sT) bf16   (ACT, no max-subtraction: |scores| <~ 6)
        AV FLIPPED: stationary = expT [128k x 128q] slices (ldweights is
            free), moving = v tile bf16 [128 x 64] -> av2[q, d] in PSUM.
            49.9k PE cycles vs 98.3k for the moving-expT orientation.
        sums = expT^T @ ones  (ap-1 matmuls into a shared [128, 96] bank)
        All sub-bank accumulation regions share one PSUM zero-region, so
        banks are Pool-memset to 0 and every matmul uses start=False with
        skip_group_check (HW accumulates onto existing content).
    normalize: r = 1/sums per (q, head) on DVE; Pool scale-copies
        av2 -> avsb bf16 (per-partition scalar = per-query, native)
    transpose avsb [q, chan] -> avT [chan, q] via XBAR DMA transpose
        (14ns per 16x128 tile; no PE cost)
    out = attnout @ Wp(bf16) + bp, K=128 accumulation over pairs

Engine budget (TimelineSim model): PE ~309k cycles @2.4GHz ~= 129us is the
bottleneck; ACT exp ~100us; DVE (rope) ~57us; Pool (copies/normalize)
~33us; DMA ~49us. Next-pair projections interleave into the j-loops so PE
never starves behind the ACT exp stream; AV emission lags scores by 2
j-tiles so PE never head-blocks on an exp dependency (engine queues are
strictly in-order).
"""

import numpy as np

import concourse.bass as bass
import concourse.mybir as mybir
import concourse.tile as tile
from concourse import bacc
from concourse.bass import ts
from concourse.bass_utils import run_bass_kernel_spmd

F32 = mybir.dt.float32
F32R = mybir.dt.float32r
BF16 = mybir.dt.bfloat16
AF = mybir.ActivationFunctionType

HEAD_NUM = 12
EMB = 768
HEAD = 64
L = 1024
B = 8
X_SIZE = 32
Y_SIZE = 32
BASE = 100.0
N_CORES = 8

KT = EMB // 128   # 6 contraction tiles over channels
NJ = L // 128     # 8 key tiles
NPAIR = HEAD_NUM // 2  # 6 head pairs


def _round_f32r(x):
    """Round fp32 to FP22 (e8m13, drop 10 mantissa LSBs, RNE)."""
    v = np.ascontiguousarray(x, dtype=np.float32).view(np.uint32).copy()
    v = v + (np.uint32(0x1FF) + ((v >> np.uint32(10)) & np.uint32(1)))
    v &= np.uint32(0xFFFFFC00)
    return v.view(np.float32)


def _tables_np(pos_len, d, base=BASE):
    inv_freq = 1.0 / base ** (np.arange(0, d, 2, dtype=np.float32) / d)
    freqs = np.outer(np.arange(pos_len, dtype=np.float32), inv_freq)
    freqs = np.concatenate([freqs, freqs], axis=-1)
    return np.sin(freqs).astype(np.float32), np.cos(freqs).astype(np.float32)


def _rope_coeffs(pos):
    """cos128/ssh128: [128, L] elementwise RoPE coefficients, 2 heads deep.

    Row layout per 64-row head block: rows 0:32 x-part, rows 32:64 y-part.
    ssh is the sin table pre-shifted/negated so that
        rope(q) = q * cos128 + R128 @ (q * ssh128)
    where R128 swaps 16-row halves within each 32-row block.
    """
    sx, cx = _tables_np(X_SIZE, HEAD // 2)
    sy, cy = _tables_np(Y_SIZE, HEAD // 2)
    px, py = pos[:, 0], pos[:, 1]
    cosxT = cx[px].T  # [32, L]
    cosyT = cy[py].T
    sinxT = sx[px].T
    sinyT = sy[py].T

    def shift(s):
        out = np.empty_like(s)
        out[0:16] = s[16:32]
        out[16:32] = -s[0:16]
        return out

    cos64 = np.concatenate([cosxT, cosyT], axis=0)          # [64, L]
    ssh64 = np.concatenate([shift(sinxT), shift(sinyT)], axis=0)
    cos128 = np.concatenate([cos64, cos64], axis=0).astype(np.float32)
    ssh128 = np.concatenate([ssh64, ssh64], axis=0).astype(np.float32)
    return np.ascontiguousarray(cos128), np.ascontiguousarray(ssh128)


def _r128():
    r32 = np.zeros((32, 32), dtype=np.float32)
    for d in range(16):
        r32[d, d + 16] = 1.0
        r32[d + 16, d] = 1.0
    return np.kron(np.eye(4, dtype=np.float32), r32)


def build_nc(debug=False):
    nc = bacc.Bacc()
    embT = nc.declare_dram_parameter("embT", [EMB, L], F32R, isOutput=False)
    wqs = nc.declare_dram_parameter("wqs", [NPAIR, 128, EMB], F32R,
                                    isOutput=False)
    wks = nc.declare_dram_parameter("wks", [NPAIR, 128, EMB], F32R,
                                    isOutput=False)
    wv = nc.declare_dram_parameter("wv", [EMB, EMB], F32R, isOutput=False)
    wp = nc.declare_dram_parameter("wp", [EMB, EMB], BF16, isOutput=False)
    bp = nc.declare_dram_parameter("bp", [1, EMB], F32, isOutput=False)
    cos = nc.declare_dram_parameter("cos", [128, L], F32, isOutput=False)
    ssh = nc.declare_dram_parameter("ssh", [128, L], F32, isOutput=False)
    r128 = nc.declare_dram_parameter("r128", [128, 128], F32R, isOutput=False)
    ident = nc.declare_dram_parameter("ident", [128, 128], BF16,
                                      isOutput=False)
    out = nc.declare_dram_parameter("out", [L, EMB], F32, isOutput=True)
    if debug:
        d_qT = nc.declare_dram_parameter("d_qT", [128, L], F32, isOutput=True)
        d_kT = nc.declare_dram_parameter("d_kT", [128, L], F32, isOutput=True)
        d_exp = nc.declare_dram_parameter("d_exp", [128, L], BF16,
                                          isOutput=True)
        d_vaug = nc.declare_dram_parameter("d_vaug", [128, EMB], BF16,
                                           isOutput=True)
        d_avsb = nc.declare_dram_parameter("d_avsb", [128, 4, 128], BF16,
                                           isOutput=True)
        d_avsb2 = nc.declare_dram_parameter("d_avsb2", [128, 4, 128], BF16,
                                            isOutput=True)
        d_avT0h1 = nc.declare_dram_parameter("d_avT0h1", [128, 4, 128], BF16,
                                             isOutput=True)
        d_rsb = nc.declare_dram_parameter("d_rsb", [128, 8], F32,
                                          isOutput=True)
        d_avT = nc.declare_dram_parameter("d_avT", [NPAIR, 128, NJ, 128],
                                          BF16, isOutput=True)
        d_sums = nc.declare_dram_parameter("d_sums", [128, 96], F32,
                                           isOutput=True)
        d_rsb2 = nc.declare_dram_parameter("d_rsb2", [128, 8], F32,
                                           isOutput=True)
        d_av2 = nc.declare_dram_parameter("d_av2", [128, 512], F32,
                                          isOutput=True)

    with tile.TileContext(nc) as tc:
        with (
            tc.tile_pool(name="const", bufs=1) as p_const,
            tc.tile_pool(name="vaug", bufs=1) as p_vaug,
            tc.tile_pool(name="persist", bufs=1) as p_per,
            tc.tile_pool(name="wsl", bufs=2) as p_wsl,
            tc.tile_pool(name="qk", bufs=2) as p_qk,
            tc.tile_pool(name="tmp", bufs=2) as p_tmp,
            tc.tile_pool(name="exp", bufs=4) as p_exp,
            tc.tile_pool(name="avsb", bufs=2) as p_avsb,
            tc.tile_pool(name="rsb", bufs=2) as p_rsb,
            tc.tile_pool(name="outp", bufs=2) as p_out,
            tc.tile_pool(name="big", bufs=2, space="PSUM") as ps_big,
            tc.tile_pool(name="qp", bufs=1, space="PSUM") as ps_qp,
            tc.tile_pool(name="av", bufs=2, space="PSUM") as ps_av,
            tc.tile_pool(name="sum", bufs=1, space="PSUM") as ps_sum,
        ):
            # attnout^T per pair: [chan 128, qtile 8, q 128] bf16.
            # Allocated FIRST so the XBAR-transpose destinations sit at the
            # lowest SBUF addresses.
            avT_t = [p_per.tile([128, NJ, 128], BF16, tag=f"avT{p}",
                                name=f"avT{p}") for p in range(NPAIR)]

            # weight slices are pre-swizzled on the host to the exact
            # [128, KT*128] SBUF layout, so each load is one contiguous DMA
            def load_wslice(w_dram, pair, wtag):
                wsl = p_wsl.tile([128, EMB], F32R, tag=wtag,
                                 name=f"wsl{wtag}{pair}")
                nc.sync.dma_start(wsl[:], w_dram[pair])
                return wsl

            # ---- loads: pair-0 q slice + embT first, then wv (v-proj is
            # interleaved into pair 0's first j-loop), rope coeffs last of
            # the startup-critical set ----
            PRE_Q0 = load_wslice(wqs, 0, "q")

            # PE warmup: throwaway matmuls during the DMA head keep the HAM
            # clock-gate warm; lands in an av-pool slot (group closes, the
            # slot is memset before real use)
            wu = p_const.tile([128, 512], F32, tag="warm")
            nc.gpsimd.memset(wu[:], 0.0)
            wup = ps_av.tile([128, 512], F32, tag="av2", name="warmps")
            for _ in range(2):
                nc.tensor.matmul(wup[0:64, :], wu[:, 0:64], wu[:],
                                 start=True, stop=True)

            embT_t = [p_per.tile([128, L], F32R, tag=f"embT{k}",
                                 name=f"embTt{k}") for k in range(KT)]
            for k in range(KT):
                eng = nc.sync if k % 2 == 0 else nc.scalar
                eng.dma_start(embT_t[k][:], embT[ts(k, 128), :])
            PRE_K0 = load_wslice(wks, 0, "k")
            # rope coefficients BEFORE wv: the q/k rope gates the first
            # scores; v-projection starts later anyway
            cos_t = p_const.tile([128, L], F32, tag="cos")
            ssh_t = p_const.tile([128, L], F32, tag="ssh")
            r_t = p_const.tile([128, 128], F32R, tag="r128")
            nc.sync.dma_start(cos_t[:], cos[:])
            nc.sync.dma_start(ssh_t[:], ssh[:])
            nc.sync.dma_start(r_t[:], r128[:])
            wv_t = [p_per.tile([128, EMB], F32R, tag=f"wvp{k}",
                               name=f"wvt{k}") for k in range(KT)]
            for k in range(KT):
                nc.sync.dma_start(wv_t[k][:], wv[ts(k, 128), :])

            # ones moving-vector for the ap-1 softmax-denominator matmuls
            ones_mv = p_const.tile([128, 1], BF16, tag="ones")
            nc.gpsimd.memset(ones_mv[:], 1.0)
            # bf16 identity for PE transposes
            id_t = p_const.tile([128, 128], BF16, tag="ident")
            nc.sync.dma_start(id_t[:], ident[:])

            # one shared PSUM bank of [128, 1] denominator accumulators,
            # region col = pair*16 + half*8 + qs*2 + head; zeroed once
            sums_ps = ps_sum.tile([128, 96], F32, tag="sums")
            nc.vector.memset(sums_ps[:], 0.0)

            vaug_t = [p_vaug.tile([128, EMB], BF16, tag=f"vaug{j}",
                                  name=f"vaug{j}")
                      for j in range(NJ)]

            def rope_into(dst, src, c0):
                """dst[:, c0:c0+512] = rope(src chunk); src is a PSUM chunk
                that gets overwritten in place by the rotation matmul."""
                t_s = p_tmp.tile([128, 512], F32R, tag="ts")
                t_c = p_tmp.tile([128, 512], F32, tag="tc")
                nc.vector.tensor_mul(t_s[:], src, ssh_t[:, c0:c0 + 512])
                nc.vector.tensor_mul(t_c[:], src, cos_t[:, c0:c0 + 512])
                nc.tensor.matmul(src, r_t[:], t_s[:], start=True, stop=True)
                nc.vector.tensor_add(dst[:, c0:c0 + 512], t_c[:], src)

            # rope'd projection, emission split into per-chunk matmul and
            # rope parts so the rotation never head-blocks the PE queue
            def make_proj(pair, wtag, wsl):
                dst = p_qk.tile([128, L], F32R, tag=wtag,
                                name=f"{wtag}T{pair}")
                qps = {}

                def mm(ci):
                    c0 = ci * 512
                    qp = ps_qp.tile([128, 512], F32, tag="qp",
                                    name=f"qp{wtag}{pair}{ci}")
                    qps[ci] = qp
                    for k in range(KT):
                        nc.tensor.matmul(
                            qp[:],
                            wsl[:, ts(k, 128)],
                            embT_t[k][:, c0:c0 + 512],
                            start=(k == 0), stop=(k == KT - 1),
                        )

                def rope(ci):
                    rope_into(dst, qps[ci][:], ci * 512)
                return dst, mm, rope

            # pair-0 q/k projections run dense through the big PSUM pool
            # (the chunked single-bank path would stall PE on rope waits
            # before there is any attention work to interleave)
            def proj_pair0(wtag, wsl):
                dst = p_qk.tile([128, L], F32R, tag=wtag, name=f"{wtag}T0")
                qp01 = ps_big.tile([128, L], F32, tag="big",
                                   name=f"qp0{wtag}")
                for c0 in (0, 512):
                    for k in range(KT):
                        nc.tensor.matmul(
                            qp01[:, c0:c0 + 512],
                            wsl[:, ts(k, 128)],
                            embT_t[k][:, c0:c0 + 512],
                            start=(k == 0), stop=(k == KT - 1),
                        )
                for c0 in (0, 512):
                    rope_into(dst, qp01[:, c0:c0 + 512], c0)
                return dst

            # v projection for one key tile -> vaug[j] (bf16); channel
            # layout already matches the flipped-AV moving operand
            def project_v(j):
                vp = ps_big.tile([128, L], F32, tag="big", name=f"vp{j}")
                for c0, c1 in ((0, 512), (512, 768)):
                    for k in range(KT):
                        nc.tensor.matmul(
                            vp[:, c0:c1],
                            embT_t[k][:, ts(j, 128)],
                            wv_t[k][:, c0:c1],
                            start=(k == 0), stop=(k == KT - 1),
                        )
                nc.vector.tensor_copy(vaug_t[j][:], vp[:, 0:EMB])

            qT = proj_pair0("q", PRE_Q0)
            kT = proj_pair0("k", PRE_K0)

            # remaining weight loads (after the startup-critical set)
            wp_t = [p_per.tile([128, EMB], BF16, tag=f"wpp{k}",
                               name=f"wpt{k}") for k in range(KT)]
            for k in range(KT):
                nc.sync.dma_start(wp_t[k][:], wp[ts(k, 128), :])
            bpb_t = p_const.tile([128, EMB], F32, tag="bpb")
            nc.sync.dma_start(bpb_t[:], bp[:].to_broadcast((128, EMB)))

            # final projection for one 128-query tile (+bias, store)
            def fp_qtile(qt):
                fp = ps_big.tile([128, L], F32, tag="big", name=f"fp{qt}")
                for c0, c1 in ((0, 512), (512, 768)):
                    for pp in range(NPAIR):
                        nc.tensor.matmul(
                            fp[:, c0:c1],
                            avT_t[pp][:, qt],
                            wp_t[pp][:, c0:c1],
                            start=(pp == 0), stop=(pp == NPAIR - 1),
                        )
                o_sb = p_out.tile([128, EMB], F32, tag="osb",
                                  name=f"osb{qt}")
                nc.vector.tensor_add(o_sb[:], fp[:, 0:EMB], bpb_t[:])
                oeng = nc.sync if qt % 2 == 0 else nc.scalar
                oeng.dma_start(out[ts(qt, 128), :], o_sb[:])

            pending_norm = None
            for pair in range(NPAIR):
                if pair + 1 < NPAIR:
                    wsl_q = load_wslice(wqs, pair + 1, "q")
                    wsl_k = load_wslice(wks, pair + 1, "k")
                    qT_n, qn_mm, qn_rope = make_proj(pair + 1, "q", wsl_q)
                    kT_n, kn_mm, kn_rope = make_proj(pair + 1, "k", wsl_k)
                for half in (0, 1):
                    av2 = ps_av.tile([128, 512], F32, tag="av2",
                                     name=f"av2_{pair}_{half}")
                    nc.vector.memset(av2[:], 0.0)

                    # flipped AV + denominator matmuls for key tile j
                    def emit_av(j, expt):
                        for qs in range(4):
                            for hh in range(2):
                                e_sl = expt[:, 512 * hh + 128 * qs:
                                            512 * hh + 128 * qs + 128]
                                nc.tensor.matmul(
                                    av2[:, qs * 128 + hh * 64:
                                        qs * 128 + hh * 64 + 64],
                                    e_sl,
                                    vaug_t[j][:, (2 * pair + hh) * 64:
                                              (2 * pair + hh) * 64 + 64],
                                    start=False, stop=(j == NJ - 1),
                                    skip_group_check=True,
                                )
                                sc = pair * 16 + half * 8 + qs * 2 + hh
                                nc.tensor.matmul(
                                    sums_ps[:, sc:sc + 1],
                                    e_sl,
                                    ones_mv[:],
                                    start=False, stop=(j == NJ - 1),
                                    skip_group_check=True,
                                )

                    pend = []
                    for j in range(NJ):
                        if len(pend) >= 2:
                            emit_av(*pend.pop(0))
                        if pair == 0 and half == 0:
                            project_v(j)
                        sAB = ps_big.tile([128, L], F32, tag="big",
                                          name=f"s{pair}_{half}_{j}")
                        for hh in range(2):
                            p0 = 64 * hh
                            nc.tensor.matmul(
                                sAB[:, 512 * hh:512 * hh + 512],
                                kT[p0:p0 + 64, ts(j, 128)],
                                qT[p0:p0 + 64, 512 * half:512 * half + 512],
                                start=True, stop=True,
                                tile_position=(p0, 0),
                            )
                        expt = p_exp.tile([128, L], BF16, tag="expt",
                                          name=f"e{pair}_{half}_{j}")
                        nc.scalar.activation(expt[:], sAB[:], AF.Exp)
                        if debug and pair == 0 and half == 0 and j == 0:
                            nc.sync.dma_start(d_exp[:], expt[:])
                            nc.sync.dma_start(d_vaug[:], vaug_t[0][:])
                        pend.append((j, expt))
                        # previous half's transposes land here, after the
                        # DVE normalize chain has had time to drain
                        if j == 1 and pending_norm is not None:
                            pending_norm()
                            pending_norm = None
                        # interleave next-pair projection chunks (q during
                        # half 0, k during half 1) so the PE excursion lands
                        # while ACT still has queued exps; the rope part is
                        # emitted one j later so PE never waits on the DVE
                        # coefficient muls
                        if pair + 1 < NPAIR:
                            mmf, ropef = ((qn_mm, qn_rope) if half == 0
                                          else (kn_mm, kn_rope))
                            if j == 2:
                                mmf(0)
                            elif j == 3:
                                ropef(0)
                            elif j == 5:
                                mmf(1)
                            elif j == 6:
                                ropef(1)
                        # last pair, half 1: interleave the final projection
                        # of the ready half-0 query tiles
                        if pair == NPAIR - 1 and half == 1 and j in (1, 3, 5, 7):
                            fp_qtile(j // 2)
                    for item in pend:
                        emit_av(*item)
                    pend.clear()

                    # normalize: per-(query, head) reciprocal, then Pool
                    # scale-copies into the bf16 transpose staging tile
                    soff = pair * 16 + half * 8
                    r_sb = p_rsb.tile([128, 8], F32, tag="rsb",
                                      name=f"r{pair}_{half}")
                    nc.vector.reciprocal_approx_fast(
                        r_sb[:], sums_ps[:, soff:soff + 8])
                    avsb = p_avsb.tile([128, 4, 128], BF16, tag="avsb",
                                       name=f"avsb{pair}_{half}")
                    for qs in range(4):
                        for hh in range(2):
                            nc.vector.tensor_scalar_mul(
                                avsb[:, qs, hh * 64:hh * 64 + 64],
                                av2[:, qs * 128 + hh * 64:
                                    qs * 128 + hh * 64 + 64],
                                r_sb[:, qs * 2 + hh:qs * 2 + hh + 1])
                    if debug and pair == 0 and half == 0:
                        nc.sync.dma_start(d_avsb[:], avsb[:])
                        nc.sync.dma_start(d_rsb[:], r_sb[:])
                        nc.scalar.dma_start(d_qT[:], qT[:].bitcast(F32))
                        nc.scalar.dma_start(d_kT[:], kT[:].bitcast(F32))
                    if debug and pair == 1 and half == 0:
                        nc.sync.dma_start(d_rsb2[:], r_sb[:])
                        av2c = p_const.tile([128, 512], F32, tag="dav2")
                        nc.vector.tensor_copy(av2c[:], av2[:])
                        nc.sync.dma_start(d_av2[:], av2c[:])
                        sumsc = p_const.tile([128, 96], F32, tag="dsums")
                        nc.vector.tensor_copy(sumsc[:], sums_ps[:])
                        nc.sync.dma_start(d_sums[:], sumsc[:])
                    # PE transpose [q, chan] -> [chan, q] per qtile (the
                    # XBAR DMA transpose corrupts under concurrent DMA
                    # traffic), then drain PSUM -> avT in SBUF. Deferred
                    # into the next half's j-loop so the PE queue never
                    # head-blocks on the DVE normalize chain; the final
                    # half drains via ACT (idle by then) immediately.
                    def make_transposes(pair, half, avsb, last):
                        def run():
                            tp = ps_big.tile([128, L], F32, tag="big",
                                             name=f"tp{pair}_{half}")
                            for qs in range(4):
                                sub = tp[:, 128 * qs:
                                         128 * qs + 64].bitcast(BF16)
                                nc.tensor.transpose(sub, avsb[:, qs, :],
                                                    id_t[:])
                                dst = avT_t[pair][:, 4 * half + qs, :]
                                if last:
                                    nc.scalar.copy(dst, sub)
                                else:
                                    nc.vector.tensor_copy(dst, sub)
                        return run

                    last = (pair == NPAIR - 1 and half == 1)
                    pending_norm = make_transposes(pair, half, avsb, last)
                    if last:
                        pending_norm()
                        pending_norm = None
                if pair + 1 < NPAIR:
                    qT, kT = qT_n, kT_n

            for qt in range(4, NJ):
                fp_qtile(qt)
            if debug:
                for p in range(NPAIR):
                    nc.sync.dma_start(d_avT[p], avT_t[p][:])

    nc.finalize()
    return nc


_NC_CACHE = {}


def _get_nc(variant=None):
    if variant not in _NC_CACHE:
        _NC_CACHE[variant] = build_nc(debug=(variant == "debug"))
    return _NC_CACHE[variant]


def kernel(emb, pos, Wq, Wk, Wv, Wp, bp, _trace=False, _cores=N_CORES,
           _debug=False):
    import ml_dtypes

    emb = np.asarray(emb, dtype=np.float32)
    pos = np.asarray(pos)
    Wq_s = _round_f32r(np.asarray(Wq, dtype=np.float32) * (HEAD ** -0.5))
    Wk_r = _round_f32r(np.asarray(Wk, dtype=np.float32))
    Wv_r = _round_f32r(np.asarray(Wv, dtype=np.float32))
    Wp_b = np.asarray(Wp, dtype=np.float32).astype(ml_dtypes.bfloat16)
    bp2 = np.asarray(bp, dtype=np.float32).reshape(1, EMB)

    cos128, ssh128 = _rope_coeffs(np.asarray(pos))
    r128 = _r128()
    ident = np.eye(128, dtype=np.float32).astype(ml_dtypes.bfloat16)

    def swizzle(w):
        # [EMB, EMB] -> [NPAIR, 128, KT*128]: slice pair columns, gather
        # row t*128+p to partition p, k-tile-major free layout
        return np.ascontiguousarray(
            w.reshape(KT, 128, NPAIR, 128).transpose(2, 1, 0, 3)
            .reshape(NPAIR, 128, EMB))

    nc = _get_nc("debug" if _debug else None)
    wqs = swizzle(Wq_s)
    wks = swizzle(Wk_r)
    in_maps = []
    for b in range(_cores):
        in_maps.append({
            "embT": _round_f32r(emb[b].T),
            "wqs": wqs, "wks": wks, "wv": Wv_r, "wp": Wp_b, "bp": bp2,
            "cos": cos128, "ssh": ssh128, "r128": r128, "ident": ident,
        })
    res = run_bass_kernel_spmd(nc, in_maps, list(range(_cores)), trace=_trace)
    out = np.stack([res.results[b]["out"] for b in range(_cores)], axis=0)
    if _debug:
        return out, res.results
    if _trace:
        return out, res
    return out
